# revision 6
# baseline (speedup 1.0000x reference)
"""Trainium2 Bass kernel for nn_Block_59115929862601.

Block: y = x + proj(attn(LN1(x))); out = y + ChebyKAN2(ChebyKAN1(LN2(y)))

Strategy (8 NeuronCores, data-parallel over batch, 4 batches/core):
- All activations live FEATURE-MAJOR on-chip ([feature, token]) so every
  matmul contraction dim is on partitions with no on-chip transposes.
  The host transposes x per core and transposes the output back.
- ChebyKAN's cos(d*arccos(tanh(h))) == T_d(tanh(h)) (Chebyshev recurrence),
  so the basis is tanh + cheap polynomials; the d=0 plane folds into a
  host-precomputed bias. Each cheby layer is then a plain matmul with
  contraction over (d, feature).
- Big matmuls (qkv-proj residual path, cheby) run in float32r (fp22): full
  PE rate, ~1e-4 error. The ISA restricts fp32r to 128-wide stationary
  free dim and even moving dim, so attention's irregular shapes
  (M=64/69/1, N=197) run in float16 instead.
- LayerNorm mean/var are partition-dim reductions done on the PE with an
  all-ones 128x128 stationary matrix, which lands the result already
  broadcast across partitions.
- Softmax runs on transposed scores (keys on partitions) without max
  subtraction (logits ~ N(0,1), exp can't overflow); denominators via
  ones-matrix PE column sums, normalization applied to e^T.
"""

from contextlib import ExitStack

import numpy as np

# ---------------- constants ----------------
B, N, C, NH, HD, HID = 32, 197, 768, 12, 64, 3072
NCORES = 8
BL = B // NCORES          # 4 local batches per core
TOK = BL * N              # 788 tokens per core
CT = C // 128             # 6 c-tiles
TC0 = (TOK + 1) // 2      # 394 token chunk
TCH = ((0, TC0), (TC0, TOK - TC0))
NG = 4                    # hidden groups for cheby streaming
GT = (HID // NG) // 128   # 6 hidden tiles per group
EPS = 1e-5
INV_C = 1.0 / C
SCALE = HD ** -0.5


def build_nc(debug=False):
    import concourse.mybir as mybir
    import concourse.tile as tile
    from concourse import bacc

    F32 = mybir.dt.float32
    F32R = mybir.dt.float32r
    F16 = mybir.dt.float16
    AF = mybir.ActivationFunctionType
    OP = mybir.AluOpType

    nc = bacc.Bacc(None, target_bir_lowering=False)

    # ---------------- DRAM I/O ----------------
    xT_d = nc.dram_tensor("xT", [C, TOK], F32R, kind="ExternalInput")
    ones_d = nc.dram_tensor("ones", [128, 128], F32R, kind="ExternalInput")
    wqkv_d = nc.dram_tensor("wqkv", [C, 3 * C], F16, kind="ExternalInput")
    bqk_d = nc.dram_tensor("bqk", [128, 12], F32, kind="ExternalInput")
    bv_d = nc.dram_tensor("bv", [1, C], F32R, kind="ExternalInput")
    wproj_d = nc.dram_tensor("wproj", [C, C], F32R, kind="ExternalInput")
    bproj_d = nc.dram_tensor("bproj", [128, CT], F32, kind="ExternalInput")
    g2_d = nc.dram_tensor("g2", [128, CT], F32, kind="ExternalInput")
    b2_d = nc.dram_tensor("b2", [128, CT], F32, kind="ExternalInput")
    # c1e[g, m, p, (k q)]: k = (d-1)*6 + j ; rows i = j*128+p ; cols o = g*768+m*128+q
    c1e_d = nc.dram_tensor("c1e", [NG, GT, 128, 18 * 128], F32R, kind="ExternalInput")
    b1e_d = nc.dram_tensor("b1e", [128, HID // 128], F32, kind="ExternalInput")
    # c2e[g, m2, p, (k q)]: k = (d-1)*6 + j ; rows i2 = g*768 + j*128 + p ; cols o = m2*128+q
    c2e_d = nc.dram_tensor("c2e", [NG, CT, 128, 18 * 128], F32R, kind="ExternalInput")
    b2e_d = nc.dram_tensor("b2e", [128, CT], F32, kind="ExternalInput")
    outT_d = nc.dram_tensor("outT", [C, TOK], F32R, kind="ExternalOutput")
    dbg = {}
    if debug:
        for name, shape in [("h1_dbg", [C, TOK]), ("qk_dbg", [2 * C, TOK]),
                            ("attn_dbg", [C, TOK]), ("y_dbg", [C, TOK]),
                            ("t1_dbg", [C, TOK]), ("hmid_dbg", [HID, TOK])]:
            dt = F16 if name in ("h1_dbg", "qk_dbg") else F32R
            dbg[name] = nc.dram_tensor(name, shape, dt, kind="ExternalOutput")

    with tile.TileContext(nc) as tc, ExitStack() as top:
        # float32r is bit-identical to float32 in SBUF; it only changes the
        # PE's read mode. The low-precision gate keys on dtype != float32.
        top.enter_context(nc.allow_low_precision(reason="float32r is 4-byte fp32"))
        const = top.enter_context(tc.tile_pool(name="const", bufs=1))
        ones_mat = const.tile([128, 128], F32R)     # all-ones stationary (LN sums)
        nc.sync.dma_start(ones_mat, ones_d[:])
        ones_row = ones_mat[0:1, :]                 # K=1 broadcast weights
        ones_f16 = const.tile([128, 128], F16)      # fp16 twin (softmax sums)
        nc.vector.memset(ones_f16, 1.0)
        eps_col = const.tile([128, 1], F32)
        nc.vector.memset(eps_col, EPS)
        bqk_sb = const.tile([128, 12], F32)
        nc.sync.dma_start(bqk_sb, bqk_d[:])
        bv_sb = const.tile([1, C], F32R)
        nc.sync.dma_start(bv_sb, bv_d[:])
        bproj_sb = const.tile([128, CT], F32)
        nc.sync.dma_start(bproj_sb, bproj_d[:])
        g2_sb = const.tile([128, CT], F32)
        nc.sync.dma_start(g2_sb, g2_d[:])
        b2_sb = const.tile([128, CT], F32)
        nc.sync.dma_start(b2_sb, b2_d[:])
        b1e_sb = const.tile([128, HID // 128], F32)
        nc.sync.dma_start(b1e_sb, b1e_d[:])
        b2e_sb = const.tile([128, CT], F32)
        nc.sync.dma_start(b2e_sb, b2e_d[:])

        big = top.enter_context(tc.tile_pool(name="big", bufs=1))
        xT = big.tile([128, CT, TOK], F32R)         # x / later y (residual)
        nc.sync.dma_start(xT, xT_d.rearrange("(k p) t -> p k t", p=128))

        # ============ LN helper (feature-major layernorm) ============
        def layernorm_bcast(src, work, ps_pool, pfx):
            """Per-token a=rstd, b=-mean*rstd of src [128, CT, TOK], already
            broadcast across partitions: returns (a_bc, b_bc) [128, TOK]."""
            ps_m = [ps_pool.tile([128, TC0], F32, tag=f"{pfx}m{ch}", bufs=1,
                                 name=f"{pfx}psm{ch}") for ch in range(2)]
            ps_q = [ps_pool.tile([128, TC0], F32, tag=f"{pfx}q{ch}", bufs=1,
                                 name=f"{pfx}psq{ch}") for ch in range(2)]
            for k in range(CT):
                sq = work.tile([128, TOK], F32R, tag="lnsq", bufs=2, name=f"{pfx}sq{k}")
                nc.vector.tensor_mul(sq, src[:, k, :], src[:, k, :])
                for ch, (t0, tl) in enumerate(TCH):
                    nc.tensor.matmul(ps_m[ch][:, :tl], ones_mat, src[:, k, t0:t0 + tl],
                                     start=(k == 0), stop=(k == CT - 1))
                    nc.tensor.matmul(ps_q[ch][:, :tl], ones_mat, sq[:, t0:t0 + tl],
                                     start=(k == 0), stop=(k == CT - 1))
            m_bc = work.tile([128, TOK], F32R, name=f"{pfx}mbc")   # mean
            a_bc = work.tile([128, TOK], F32R, name=f"{pfx}abc")   # rstd
            b_bc = work.tile([128, TOK], F32R, name=f"{pfx}bbc")   # -mean*rstd
            for ch, (t0, tl) in enumerate(TCH):
                nc.vector.tensor_scalar_mul(m_bc[:, t0:t0 + tl], ps_m[ch][:, :tl], INV_C)
                nc.vector.tensor_scalar_mul(a_bc[:, t0:t0 + tl], ps_q[ch][:, :tl], INV_C)
            # a_bc currently holds mean(x^2); var = msq - mean^2
            nc.vector.tensor_mul(b_bc, m_bc, m_bc)
            nc.vector.tensor_tensor(a_bc, a_bc, b_bc, OP.subtract)
            nc.scalar.activation(a_bc, a_bc, AF.Sqrt, bias=eps_col)
            nc.vector.reciprocal(a_bc, a_bc)                       # rstd
            nc.vector.tensor_mul(b_bc, m_bc, a_bc)
            nc.vector.tensor_scalar_mul(b_bc, b_bc, -1.0)
            return a_bc, b_bc

        # ======== Phases 1+2 share qkT / v_tm ========
        with ExitStack() as attn_scope:
            qkp = attn_scope.enter_context(tc.tile_pool(name="qkp", bufs=1))
            qkT = qkp.tile([128, 12, TOK], F16)     # q tiles 0..5, k tiles 6..11
            vtp = attn_scope.enter_context(tc.tile_pool(name="vtp", bufs=1))
            v_tm = vtp.tile([128, BL, 2, C], F16)   # token-major V, per batch

            # ---- Phase 1: LN1 + QKV + V ----
            with ExitStack() as ph:
                work = ph.enter_context(tc.tile_pool(name="w1", bufs=1))
                h1p = ph.enter_context(tc.tile_pool(name="h1p", bufs=1))
                h1 = h1p.tile([128, CT, TOK], F16)
                with tc.tile_pool(name="ps_ln", bufs=1, space="PSUM") as ps_ln:
                    a_bc, b_bc = layernorm_bcast(xT, work, ps_ln, "ln1")
                    for k in range(CT):
                        tmp = work.tile([128, TOK], F32R, tag="h1t", bufs=2,
                                        name=f"h1t{k}")
                        nc.vector.tensor_mul(tmp, xT[:, k, :], a_bc)
                        nc.vector.tensor_add(h1[:, k, :], tmp, b_bc)
                if debug:
                    nc.sync.dma_start(
                        dbg["h1_dbg"].rearrange("(k p) t -> p k t", p=128), h1)

                wqkv_p = ph.enter_context(tc.tile_pool(name="wqkv", bufs=1))
                wqkv_sb = wqkv_p.tile([128, CT, 3 * C], F16)
                nc.sync.dma_start(wqkv_sb, wqkv_d.rearrange("(k p) m -> p k m", p=128))

                with tc.tile_pool(name="ps_mm", bufs=1, space="PSUM") as ps_mm:
                    for m in range(12):
                        for ch, (t0, tl) in enumerate(TCH):
                            ps = ps_mm.tile([128, TC0], F32, tag="qkv", bufs=3,
                                            name=f"qk{m}_{ch}")
                            for k in range(CT):
                                nc.tensor.matmul(
                                    ps[:, :tl], wqkv_sb[:, k, m * 128:(m + 1) * 128],
                                    h1[:, k, t0:t0 + tl],
                                    start=(k == 0), stop=(k == CT - 1))
                            nc.vector.tensor_scalar_add(qkT[:, m, t0:t0 + tl],
                                                        ps[:, :tl], bqk_sb[:, m:m + 1])
                    if debug:
                        nc.sync.dma_start(
                            dbg["qk_dbg"].rearrange("(k p) t -> p k t", p=128), qkT)

                    bv_bc = work.tile([128, C], F32)
                    for nchs in range(2):
                        ps_b = ps_mm.tile([128, C // 2], F32, tag="bvbc", bufs=1,
                                          name=f"bv{nchs}")
                        nc.tensor.matmul(ps_b, ones_row,
                                         bv_sb[:, nchs * 384:(nchs + 1) * 384],
                                         start=True, stop=True)
                        nc.vector.tensor_copy(bv_bc[:, nchs * 384:(nchs + 1) * 384], ps_b)

                    for b in range(BL):
                        for half, rows in ((0, 128), (1, N - 128)):
                            tok0 = b * N + half * 128
                            for nchs in range(2):
                                ps = ps_mm.tile([128, C // 2], F32, tag="vmm", bufs=2,
                                                name=f"v{b}_{half}_{nchs}")
                                nc0 = 2 * C + nchs * 384
                                for k in range(CT):
                                    nc.tensor.matmul(
                                        ps[:rows], h1[:, k, tok0:tok0 + rows],
                                        wqkv_sb[:, k, nc0:nc0 + 384],
                                        start=(k == 0), stop=(k == CT - 1))
                                nc.vector.tensor_add(
                                    v_tm[:rows, b, half, nchs * 384:(nchs + 1) * 384],
                                    ps[:rows], bv_bc[:rows, nchs * 384:(nchs + 1) * 384])

            # ---- Phase 2: attention + proj ----
            with ExitStack() as ph:
                atp = ph.enter_context(tc.tile_pool(name="atp", bufs=1))
                attnT = atp.tile([128, CT, TOK], F32R)
                ep = ph.enter_context(tc.tile_pool(name="ep", bufs=1))
                with tc.tile_pool(name="ps_at", bufs=1, space="PSUM") as ps_at:
                    for b in range(BL):
                        bsl = slice(b * N, (b + 1) * N)
                        for hp in range(NH // 2):
                            ps_av = ps_at.tile([128, N], F32, tag="av", bufs=2,
                                               name=f"av{b}_{hp}")
                            for par in range(2):
                                h = 2 * hp + par
                                po = par * 64
                                qap = qkT[po:po + 64, hp, bsl]
                                eT = [ep.tile([128, N], F16, tag=f"e{ktc}", bufs=2,
                                              name=f"e{b}_{h}_{ktc}")
                                      for ktc in range(2)]
                                ps_cs = ps_at.tile([128, N], F32, tag="cs", bufs=1,
                                                   name=f"cs{b}_{h}")
                                for ktc, rows in ((0, 128), (1, N - 128)):
                                    kap = qkT[po:po + 64, 6 + hp,
                                              b * N + ktc * 128:b * N + ktc * 128 + rows]
                                    ps_s = ps_at.tile([128, N], F32, tag="s", bufs=2,
                                                      name=f"s{b}_{h}_{ktc}")
                                    nc.tensor.matmul(ps_s[:rows], kap, qap,
                                                     start=True, stop=True)
                                    nc.scalar.activation(eT[ktc][:rows], ps_s[:rows],
                                                         AF.Exp, scale=SCALE)
                                    nc.tensor.matmul(ps_cs, ones_f16[:rows],
                                                     eT[ktc][:rows],
                                                     start=(ktc == 0), stop=(ktc == 1))
                                rbc = ep.tile([128, N], F16, tag="rbc", bufs=2,
                                              name=f"rb{b}_{h}")
                                nc.vector.reciprocal(rbc, ps_cs)
                                for ktc, rows in ((0, 128), (1, N - 128)):
                                    nc.vector.tensor_mul(eT[ktc][:rows], eT[ktc][:rows],
                                                         rbc[:rows])
                                for ktc, rows in ((0, 128), (1, N - 128)):
                                    nc.tensor.matmul(
                                        ps_av[po:po + 64],
                                        v_tm[:rows, b, ktc, h * 64:(h + 1) * 64],
                                        eT[ktc][:rows],
                                        start=(ktc == 0), stop=(ktc == 1))
                            nc.vector.tensor_copy(attnT[:, hp, bsl], ps_av)
                if debug:
                    nc.sync.dma_start(
                        dbg["attn_dbg"].rearrange("(k p) t -> p k t", p=128), attnT)

                wpp = ph.enter_context(tc.tile_pool(name="wpp", bufs=1))
                wproj_sb = wpp.tile([128, CT, C], F32R)
                nc.sync.dma_start(wproj_sb, wproj_d.rearrange("(k p) m -> p k m", p=128))
                with tc.tile_pool(name="ps_pj", bufs=1, space="PSUM") as ps_pj:
                    for m in range(CT):
                        for ch, (t0, tl) in enumerate(TCH):
                            ps = ps_pj.tile([128, TC0], F32, tag="pj", bufs=3,
                                            name=f"pj{m}_{ch}")
                            for k in range(CT):
                                nc.tensor.matmul(
                                    ps[:, :tl], wproj_sb[:, k, m * 128:(m + 1) * 128],
                                    attnT[:, k, t0:t0 + tl],
                                    start=(k == 0), stop=(k == CT - 1))
                            tmp = ep.tile([128, TC0], F32R, tag="pjt", bufs=2,
                                          name=f"pjt{m}_{ch}")
                            nc.vector.tensor_scalar_add(tmp[:, :tl], ps[:, :tl],
                                                        bproj_sb[:, m:m + 1])
                            nc.vector.tensor_add(xT[:, m, t0:t0 + tl],
                                                 xT[:, m, t0:t0 + tl], tmp[:, :tl])
                if debug:
                    nc.sync.dma_start(
                        dbg["y_dbg"].rearrange("(k p) t -> p k t", p=128), xT)

        # ============ Phase 3: LN2 + basis1 ============
        bsp = top.enter_context(tc.tile_pool(name="bsp", bufs=1))
        basis1 = bsp.tile([128, 18, TOK], F32R)     # planes: t 0..5, T2 6..11, T3 12..17
        with ExitStack() as ph:
            work = ph.enter_context(tc.tile_pool(name="w3", bufs=1))
            with tc.tile_pool(name="ps_ln2", bufs=1, space="PSUM") as ps_ln2:
                a2, b2b = layernorm_bcast(xT, work, ps_ln2, "ln2")
                for k in range(CT):
                    nsc = work.tile([128, TOK], F32R, tag="nsc", bufs=2, name=f"n{k}")
                    nc.vector.tensor_mul(nsc, xT[:, k, :], a2)
                    nc.vector.tensor_add(nsc, nsc, b2b)
                    nc.vector.tensor_scalar(nsc, nsc, g2_sb[:, k:k + 1],
                                            b2_sb[:, k:k + 1], OP.mult, OP.add)
                    nc.scalar.activation(basis1[:, k, :], nsc, AF.Tanh)
            for k in range(CT):
                t_ = basis1[:, k, :]
                p2 = basis1[:, 6 + k, :]
                p3 = basis1[:, 12 + k, :]
                nc.vector.tensor_mul(p2, t_, t_)
                nc.vector.tensor_scalar(p2, p2, 2.0, -1.0, OP.mult, OP.add)
                nc.vector.tensor_scalar(p3, p2, 2.0, -1.0, OP.mult, OP.add)
                nc.vector.tensor_mul(p3, p3, t_)
            if debug:
                nc.sync.dma_start(
                    dbg["t1_dbg"].rearrange("(k p) t -> p k t", p=128),
                    basis1[:, 0:6, :])
            for m2 in range(CT):    # y += cheby2 d=0 bias, before accumulation
                nc.vector.tensor_scalar_add(xT[:, m2, :], xT[:, m2, :],
                                            b2e_sb[:, m2:m2 + 1])

        # ============ Phase 4: cheby1 -> basis2 -> cheby2 (per hidden group) ============
        with ExitStack() as ph:
            gb = ph.enter_context(tc.tile_pool(name="gb", bufs=1))
            tg = gb.tile([128, GT, TOK], F32R)
            p2g = gb.tile([128, GT, TOK], F32R)
            p3g = gb.tile([128, GT, TOK], F32R)
            c1s = ph.enter_context(tc.tile_pool(name="c1s", bufs=2))
            c2s = ph.enter_context(tc.tile_pool(name="c2s", bufs=2))
            ps_c1 = ph.enter_context(tc.tile_pool(name="ps_c1", bufs=3, space="PSUM"))
            ps_c2 = ph.enter_context(tc.tile_pool(name="ps_c2", bufs=3, space="PSUM"))
            for g in range(NG):
                for m in range(GT):
                    cw = c1s.tile([128, 18, 128], F32R, tag="c1w", name=f"c1_{g}_{m}")
                    nc.sync.dma_start(
                        cw, c1e_d[g, m].rearrange("p (k q) -> p k q", q=128))
                    hm = g * GT + m
                    for ch, (t0, tl) in enumerate(TCH):
                        ps = ps_c1.tile([128, TC0], F32, tag="c1",
                                        name=f"pc1_{g}_{m}_{ch}")
                        for k in range(18):
                            nc.tensor.matmul(ps[:, :tl], cw[:, k, :],
                                             basis1[:, k, t0:t0 + tl],
                                             start=(k == 0), stop=(k == 17))
                        nc.scalar.activation(tg[:, m, t0:t0 + tl], ps[:, :tl], AF.Tanh,
                                             bias=b1e_sb[:, hm:hm + 1])
                if debug:
                    nc.sync.dma_start(
                        dbg["hmid_dbg"].rearrange("(g m p) t -> p g m t",
                                                  p=128, g=NG)[:, g], tg)
                for m in range(GT):
                    t_ = tg[:, m, :]
                    nc.vector.tensor_mul(p2g[:, m, :], t_, t_)
                    nc.vector.tensor_scalar(p2g[:, m, :], p2g[:, m, :], 2.0, -1.0,
                                            OP.mult, OP.add)
                    nc.vector.tensor_scalar(p3g[:, m, :], p2g[:, m, :], 2.0, -1.0,
                                            OP.mult, OP.add)
                    nc.vector.tensor_mul(p3g[:, m, :], p3g[:, m, :], t_)
                planes = (tg, p2g, p3g)
                for m2 in range(CT):
                    cw2 = c2s.tile([128, 18, 128], F32R, tag="c2w", name=f"c2_{g}_{m2}")
                    nc.sync.dma_start(
                        cw2, c2e_d[g, m2].rearrange("p (k q) -> p k q", q=128))
                    for ch, (t0, tl) in enumerate(TCH):
                        ps = ps_c2.tile([128, TC0], F32, tag="c2",
                                        name=f"pc2_{g}_{m2}_{ch}")
                        for k in range(18):
                            d, j = divmod(k, 6)
                            nc.tensor.matmul(ps[:, :tl], cw2[:, k, :],
                                             planes[d][:, j, t0:t0 + tl],
                                             start=(k == 0), stop=(k == 17))
                        nc.vector.tensor_add(xT[:, m2, t0:t0 + tl],
                                             xT[:, m2, t0:t0 + tl], ps[:, :tl])

        nc.sync.dma_start(outT_d.rearrange("(k p) t -> p k t", p=128), xT)

    nc.compile()
    return nc


# ---------------- host-side preprocessing ----------------
def _prep_weights(g1, b1, w_qkv, w_proj, b_proj, g2, b2, c1, c2):
    f32 = np.float32
    wqkv_eff = (g1[:, None] * w_qkv).astype(np.float16)          # [C, 3C] fp16
    bqkv = (b1 @ w_qkv).astype(f32)                              # [3C]
    bqk = np.ascontiguousarray(bqkv[:2 * C].reshape(12, 128).T)  # [128, 12]
    bv = np.ascontiguousarray(bqkv[2 * C:].reshape(1, C))        # [1, C]
    bproj = np.ascontiguousarray(b_proj.reshape(CT, 128).T)      # [128, CT]
    g2p = np.ascontiguousarray(g2.reshape(CT, 128).T)
    b2p = np.ascontiguousarray(b2.reshape(CT, 128).T)
    c1p = np.ascontiguousarray(c1[:, :, 1:4].transpose(2, 0, 1)) # [3, C, HID]
    c1p = c1p.reshape(3, CT, 128, NG, GT, 128)                   # d, j, p, g, m, q
    c1e = np.ascontiguousarray(c1p.transpose(3, 4, 2, 0, 1, 5))  # g, m, p, d, j, q
    c1e = c1e.reshape(NG, GT, 128, 18 * 128)
    b1e = np.ascontiguousarray(c1[:, :, 0].sum(0).reshape(HID // 128, 128).T)
    c2p = np.ascontiguousarray(c2[:, :, 1:4].transpose(2, 0, 1)) # [3, HID, C]
    c2p = c2p.reshape(3, NG, CT, 128, CT, 128)                   # d, g, j, p, m2, q
    c2e = np.ascontiguousarray(c2p.transpose(1, 4, 3, 0, 2, 5))  # g, m2, p, d, j, q
    c2e = c2e.reshape(NG, CT, 128, 18 * 128)
    b2e = np.ascontiguousarray(c2[:, :, 0].sum(0).reshape(CT, 128).T)
    return dict(ones=np.ones((128, 128), f32), wqkv=wqkv_eff, bqk=bqk, bv=bv,
                wproj=np.ascontiguousarray(w_proj), bproj=bproj, g2=g2p, b2=b2p,
                c1e=c1e, b1e=b1e, c2e=c2e, b2e=b2e)


_NC_CACHE = {}


def kernel(x, g1, b1, w_qkv, w_proj, b_proj, g2, b2, c1, c2):
    x = np.asarray(x, np.float32)
    wd = _prep_weights(np.asarray(g1, np.float32), np.asarray(b1, np.float32),
                       np.asarray(w_qkv, np.float32), np.asarray(w_proj, np.float32),
                       np.asarray(b_proj, np.float32), np.asarray(g2, np.float32),
                       np.asarray(b2, np.float32), np.asarray(c1, np.float32),
                       np.asarray(c2, np.float32))
    if "nc" not in _NC_CACHE:
        _NC_CACHE["nc"] = build_nc(debug=False)
    nc = _NC_CACHE["nc"]

    in_maps = []
    for i in range(NCORES):
        xs = x[BL * i:BL * (i + 1)].reshape(TOK, C)
        m = {"xT": np.ascontiguousarray(xs.T)}
        m.update(wd)
        in_maps.append(m)

    from concourse.bass_utils import run_bass_kernel_spmd
    res = run_bass_kernel_spmd(nc, in_maps, core_ids=list(range(NCORES)))
    outs = []
    for i in range(NCORES):
        yT = res.results[i]["outT"]                  # [C, TOK]
        outs.append(yT.T.reshape(BL, N, C))
    return np.concatenate(outs, 0).astype(np.float32)


if __name__ == "__main__":
    build_nc()
    print("built ok")


# revision 9
# speedup vs baseline: 1.0784x; 1.0784x over previous
"""Trainium2 Bass kernel for nn_Block_59115929862601.

Block: y = x + proj(attn(LN1(x))); out = y + ChebyKAN2(ChebyKAN1(LN2(y)))

Strategy (8 NeuronCores, data-parallel over batch, 4 batches/core):
- All activations live FEATURE-MAJOR on-chip ([feature, token]) so every
  matmul contraction dim is on partitions with no on-chip transposes.
  The host transposes x per core and transposes the output back.
- ChebyKAN's cos(d*arccos(tanh(h))) == T_d(tanh(h)) (Chebyshev recurrence),
  so the basis is tanh + cheap polynomials; the d=0 plane folds into a
  host-precomputed bias. Each cheby layer is then a plain matmul with
  contraction over (d, feature).
- Big matmuls (qkv-proj residual path, cheby) run in float32r (fp22): full
  PE rate, ~1e-4 error. The ISA restricts fp32r to 128-wide stationary
  free dim and even moving dim, so attention's irregular shapes
  (M=64/69/1, N=197) run in float16 instead.
- LayerNorm mean/var are partition-dim reductions done on the PE with an
  all-ones 128x128 stationary matrix, which lands the result already
  broadcast across partitions.
- Softmax runs on transposed scores (keys on partitions) without max
  subtraction (logits ~ N(0,1), exp can't overflow); denominators via
  ones-matrix PE column sums, normalization applied to e^T.
"""

from contextlib import ExitStack

import numpy as np

# ---------------- constants ----------------
B, N, C, NH, HD, HID = 32, 197, 768, 12, 64, 3072
NCORES = 8
BL = B // NCORES          # 4 local batches per core
TOK = BL * N              # 788 tokens per core
CT = C // 128             # 6 c-tiles
TC0 = (TOK + 1) // 2      # 394 token chunk
TCH = ((0, TC0), (TC0, TOK - TC0))
NG = 4                    # hidden groups for cheby streaming
GT = (HID // NG) // 128   # 6 hidden tiles per group
EPS = 1e-5
INV_C = 1.0 / C
SCALE = HD ** -0.5


def build_nc(debug=False):
    import concourse.mybir as mybir
    import concourse.tile as tile
    from concourse import bacc

    F32 = mybir.dt.float32
    F32R = mybir.dt.float32r
    F16 = mybir.dt.float16
    AF = mybir.ActivationFunctionType
    OP = mybir.AluOpType

    nc = bacc.Bacc(None, target_bir_lowering=False)

    # ---------------- DRAM I/O ----------------
    xT_d = nc.dram_tensor("xT", [C, TOK], F32R, kind="ExternalInput")
    ones_d = nc.dram_tensor("ones", [128, 128], F32R, kind="ExternalInput")
    wqkv_d = nc.dram_tensor("wqkv", [C, 3 * C], F16, kind="ExternalInput")
    bqk_d = nc.dram_tensor("bqk", [128, 12], F32, kind="ExternalInput")
    bv_d = nc.dram_tensor("bv", [1, C], F32R, kind="ExternalInput")
    wproj_d = nc.dram_tensor("wproj", [C, C], F32R, kind="ExternalInput")
    bproj_d = nc.dram_tensor("bproj", [128, CT], F32, kind="ExternalInput")
    g2_d = nc.dram_tensor("g2", [128, CT], F32, kind="ExternalInput")
    b2_d = nc.dram_tensor("b2", [128, CT], F32, kind="ExternalInput")
    # c1e[g, m, p, (k q)]: k = (d-1)*6 + j ; rows i = j*128+p ; cols o = g*768+m*128+q
    c1e_d = nc.dram_tensor("c1e", [NG, GT, 128, 18 * 128], F32R, kind="ExternalInput")
    b1e_d = nc.dram_tensor("b1e", [128, HID // 128], F32, kind="ExternalInput")
    # c2e[g, m2, p, (k q)]: k = (d-1)*6 + j ; rows i2 = g*768 + j*128 + p ; cols o = m2*128+q
    c2e_d = nc.dram_tensor("c2e", [NG, CT, 128, 18 * 128], F32R, kind="ExternalInput")
    b2e_d = nc.dram_tensor("b2e", [128, CT], F32, kind="ExternalInput")
    outT_d = nc.dram_tensor("outT", [C, TOK], F32R, kind="ExternalOutput")
    dbg = {}
    if debug:
        for name, shape in [("h1_dbg", [C, TOK]), ("qk_dbg", [2 * C, TOK]),
                            ("attn_dbg", [C, TOK]), ("y_dbg", [C, TOK]),
                            ("t1_dbg", [C, TOK]), ("hmid_dbg", [HID, TOK])]:
            dt = F16 if name in ("h1_dbg", "qk_dbg") else F32R
            dbg[name] = nc.dram_tensor(name, shape, dt, kind="ExternalOutput")

    with tile.TileContext(nc) as tc, ExitStack() as top:
        # float32r is bit-identical to float32 in SBUF; it only changes the
        # PE's read mode. The low-precision gate keys on dtype != float32.
        top.enter_context(nc.allow_low_precision(reason="float32r is 4-byte fp32"))
        const = top.enter_context(tc.tile_pool(name="const", bufs=1))
        ones_mat = const.tile([128, 128], F32R)     # all-ones stationary (LN sums)
        nc.sync.dma_start(ones_mat, ones_d[:])
        ones_row = ones_mat[0:1, :]                 # K=1 broadcast weights
        ones_f16 = const.tile([128, 128], F16)      # fp16 twin (softmax sums)
        nc.vector.memset(ones_f16, 1.0)
        eps_col = const.tile([128, 1], F32)
        nc.vector.memset(eps_col, EPS)
        bqk_sb = const.tile([128, 12], F32)
        nc.sync.dma_start(bqk_sb, bqk_d[:])
        bv_sb = const.tile([1, C], F32R)
        nc.sync.dma_start(bv_sb, bv_d[:])
        bproj_sb = const.tile([128, CT], F32)
        nc.sync.dma_start(bproj_sb, bproj_d[:])
        g2_sb = const.tile([128, CT], F32)
        nc.sync.dma_start(g2_sb, g2_d[:])
        b2_sb = const.tile([128, CT], F32)
        nc.sync.dma_start(b2_sb, b2_d[:])
        b1e_sb = const.tile([128, HID // 128], F32)
        nc.sync.dma_start(b1e_sb, b1e_d[:])
        b2e_sb = const.tile([128, CT], F32)
        nc.sync.dma_start(b2e_sb, b2e_d[:])

        big = top.enter_context(tc.tile_pool(name="big", bufs=1))
        xT = big.tile([128, CT, TOK], F32R)         # x / later y (residual)
        nc.sync.dma_start(xT, xT_d.rearrange("(k p) t -> p k t", p=128))

        # ============ LN helper (feature-major layernorm) ============
        def layernorm_bcast(src, work, ps_pool, pfx):
            """Per-token a=rstd, b=-mean*rstd of src [128, CT, TOK], already
            broadcast across partitions: returns (a_bc, b_bc) [128, TOK]."""
            ps_m = [ps_pool.tile([128, TC0], F32, tag=f"{pfx}m{ch}", bufs=1,
                                 name=f"{pfx}psm{ch}") for ch in range(2)]
            ps_q = [ps_pool.tile([128, TC0], F32, tag=f"{pfx}q{ch}", bufs=1,
                                 name=f"{pfx}psq{ch}") for ch in range(2)]
            for k in range(CT):
                sq = work.tile([128, TOK], F32R, tag="lnsq", bufs=2, name=f"{pfx}sq{k}")
                nc.vector.tensor_mul(sq, src[:, k, :], src[:, k, :])
                for ch, (t0, tl) in enumerate(TCH):
                    nc.tensor.matmul(ps_m[ch][:, :tl], ones_mat, src[:, k, t0:t0 + tl],
                                     start=(k == 0), stop=(k == CT - 1))
                    nc.tensor.matmul(ps_q[ch][:, :tl], ones_mat, sq[:, t0:t0 + tl],
                                     start=(k == 0), stop=(k == CT - 1))
            m_bc = work.tile([128, TOK], F32R, name=f"{pfx}mbc")   # mean
            a_bc = work.tile([128, TOK], F32R, name=f"{pfx}abc")   # rstd
            b_bc = work.tile([128, TOK], F32R, name=f"{pfx}bbc")   # -mean*rstd
            for ch, (t0, tl) in enumerate(TCH):
                nc.vector.tensor_scalar_mul(m_bc[:, t0:t0 + tl], ps_m[ch][:, :tl], INV_C)
                nc.vector.tensor_scalar_mul(a_bc[:, t0:t0 + tl], ps_q[ch][:, :tl], INV_C)
            # a_bc currently holds mean(x^2); var = msq - mean^2
            nc.vector.tensor_mul(b_bc, m_bc, m_bc)
            nc.vector.tensor_tensor(a_bc, a_bc, b_bc, OP.subtract)
            nc.scalar.activation(a_bc, a_bc, AF.Sqrt, bias=eps_col)
            nc.vector.reciprocal(a_bc, a_bc)                       # rstd
            nc.vector.tensor_mul(b_bc, m_bc, a_bc)
            nc.vector.tensor_scalar_mul(b_bc, b_bc, -1.0)
            return a_bc, b_bc

        # ======== Phases 1+2 share qkT / v_tm ========
        with ExitStack() as attn_scope:
            qkp = attn_scope.enter_context(tc.tile_pool(name="qkp", bufs=1))
            qkT = qkp.tile([128, 12, TOK], F16)     # q tiles 0..5, k tiles 6..11
            vtp = attn_scope.enter_context(tc.tile_pool(name="vtp", bufs=1))
            v_tm = vtp.tile([128, BL, 2, C], F16)   # token-major V, per batch

            # ---- Phase 1: LN1 + QKV + V ----
            with ExitStack() as ph:
                ph.enter_context(nc.named_scope("p1_ln1_qkv"))
                work = ph.enter_context(tc.tile_pool(name="w1", bufs=1))
                h1p = ph.enter_context(tc.tile_pool(name="h1p", bufs=1))
                h1 = h1p.tile([128, CT, TOK], F16)
                with tc.tile_pool(name="ps_ln", bufs=1, space="PSUM") as ps_ln:
                    a_bc, b_bc = layernorm_bcast(xT, work, ps_ln, "ln1")
                    for k in range(CT):
                        tmp = work.tile([128, TOK], F32R, tag="h1t", bufs=2,
                                        name=f"h1t{k}")
                        nc.vector.tensor_mul(tmp, xT[:, k, :], a_bc)
                        nc.vector.tensor_add(h1[:, k, :], tmp, b_bc)
                if debug:
                    nc.sync.dma_start(
                        dbg["h1_dbg"].rearrange("(k p) t -> p k t", p=128), h1)

                wqkv_p = ph.enter_context(tc.tile_pool(name="wqkv", bufs=1))
                wqkv_sb = wqkv_p.tile([128, CT, 3 * C], F16)
                nc.sync.dma_start(wqkv_sb, wqkv_d.rearrange("(k p) m -> p k m", p=128))

                with tc.tile_pool(name="ps_mm", bufs=1, space="PSUM") as ps_mm:
                    for m in range(12):
                        for ch, (t0, tl) in enumerate(TCH):
                            ps = ps_mm.tile([128, TC0], F32, tag="qkv", bufs=3,
                                            name=f"qk{m}_{ch}")
                            for k in range(CT):
                                nc.tensor.matmul(
                                    ps[:, :tl], wqkv_sb[:, k, m * 128:(m + 1) * 128],
                                    h1[:, k, t0:t0 + tl],
                                    start=(k == 0), stop=(k == CT - 1))
                            nc.vector.tensor_scalar_add(qkT[:, m, t0:t0 + tl],
                                                        ps[:, :tl], bqk_sb[:, m:m + 1])
                    if debug:
                        nc.sync.dma_start(
                            dbg["qk_dbg"].rearrange("(k p) t -> p k t", p=128), qkT)

                    bv_bc = work.tile([128, C], F32)
                    for nchs in range(2):
                        ps_b = ps_mm.tile([128, C // 2], F32, tag="bvbc", bufs=1,
                                          name=f"bv{nchs}")
                        nc.tensor.matmul(ps_b, ones_row,
                                         bv_sb[:, nchs * 384:(nchs + 1) * 384],
                                         start=True, stop=True)
                        nc.vector.tensor_copy(bv_bc[:, nchs * 384:(nchs + 1) * 384], ps_b)

                    for b in range(BL):
                        for half, rows in ((0, 128), (1, N - 128)):
                            tok0 = b * N + half * 128
                            for nchs in range(2):
                                ps = ps_mm.tile([128, C // 2], F32, tag="vmm", bufs=2,
                                                name=f"v{b}_{half}_{nchs}")
                                nc0 = 2 * C + nchs * 384
                                for k in range(CT):
                                    nc.tensor.matmul(
                                        ps[:rows], h1[:, k, tok0:tok0 + rows],
                                        wqkv_sb[:, k, nc0:nc0 + 384],
                                        start=(k == 0), stop=(k == CT - 1))
                                nc.vector.tensor_add(
                                    v_tm[:rows, b, half, nchs * 384:(nchs + 1) * 384],
                                    ps[:rows], bv_bc[:rows, nchs * 384:(nchs + 1) * 384])

            # ---- Phase 2: attention + proj ----
            with ExitStack() as ph:
                ph.enter_context(nc.named_scope("p2_attn_proj"))
                atp = ph.enter_context(tc.tile_pool(name="atp", bufs=1))
                attnT = atp.tile([128, CT, TOK], F32R)
                ep = ph.enter_context(tc.tile_pool(name="ep", bufs=1))
                with tc.tile_pool(name="ps_at", bufs=1, space="PSUM") as ps_at:
                    for b in range(BL):
                        bsl = slice(b * N, (b + 1) * N)
                        for hp in range(NH // 2):
                            ps_av = ps_at.tile([128, N], F32, tag="av", bufs=2,
                                               name=f"av{b}_{hp}")
                            rnorm = ep.tile([128, N], F32, tag="rnorm", bufs=2,
                                            name=f"rn{b}_{hp}")
                            csb = ep.tile([128, N], F32, tag="csb", bufs=2,
                                          name=f"csb{b}_{hp}")
                            for par in range(2):
                                h = 2 * hp + par
                                po = par * 64
                                qap = qkT[po:po + 64, hp, bsl]
                                eT = [ep.tile([128, N], F16, tag=f"e{ktc}", bufs=3,
                                              name=f"e{b}_{h}_{ktc}")
                                      for ktc in range(2)]
                                ps_cs = ps_at.tile([128, N], F32, tag="cs", bufs=2,
                                                   name=f"cs{b}_{h}")
                                for ktc, rows in ((0, 128), (1, N - 128)):
                                    kap = qkT[po:po + 64, 6 + hp,
                                              b * N + ktc * 128:b * N + ktc * 128 + rows]
                                    ps_s = ps_at.tile([128, N], F32, tag="s", bufs=2,
                                                      name=f"s{b}_{h}_{ktc}")
                                    nc.tensor.matmul(ps_s[:rows], kap, qap,
                                                     start=True, stop=True)
                                    nc.scalar.activation(eT[ktc][:rows], ps_s[:rows],
                                                         AF.Exp, scale=SCALE)
                                    nc.tensor.matmul(ps_cs, ones_f16[:rows],
                                                     eT[ktc][:rows],
                                                     start=(ktc == 0), stop=(ktc == 1))
                                # stage this head's softmax denominators into the
                                # pair's csb half (custom-DVE reciprocal needs SBUF
                                # source at partition base 0, so run it full-width
                                # once per pair below)
                                nc.vector.tensor_copy(csb[po:po + 64],
                                                      ps_cs[po:po + 64])
                                for ktc, rows in ((0, 128), (1, N - 128)):
                                    nc.tensor.matmul(
                                        ps_av[po:po + 64],
                                        v_tm[:rows, b, ktc, h * 64:(h + 1) * 64],
                                        eT[ktc][:rows],
                                        start=(ktc == 0), stop=(ktc == 1))
                            nc.vector.reciprocal_approx_fast(rnorm, csb)
                            nc.vector.tensor_mul(attnT[:, hp, bsl], ps_av, rnorm)
                if debug:
                    nc.sync.dma_start(
                        dbg["attn_dbg"].rearrange("(k p) t -> p k t", p=128), attnT)

                wpp = ph.enter_context(tc.tile_pool(name="wpp", bufs=1))
                wproj_sb = wpp.tile([128, CT, C], F32R)
                nc.sync.dma_start(wproj_sb, wproj_d.rearrange("(k p) m -> p k m", p=128))
                with tc.tile_pool(name="ps_pj", bufs=1, space="PSUM") as ps_pj:
                    for m in range(CT):
                        for ch, (t0, tl) in enumerate(TCH):
                            ps = ps_pj.tile([128, TC0], F32, tag="pj", bufs=3,
                                            name=f"pj{m}_{ch}")
                            for k in range(CT):
                                nc.tensor.matmul(
                                    ps[:, :tl], wproj_sb[:, k, m * 128:(m + 1) * 128],
                                    attnT[:, k, t0:t0 + tl],
                                    start=(k == 0), stop=(k == CT - 1))
                            tmp = ep.tile([128, TC0], F32R, tag="pjt", bufs=2,
                                          name=f"pjt{m}_{ch}")
                            nc.vector.tensor_scalar_add(tmp[:, :tl], ps[:, :tl],
                                                        bproj_sb[:, m:m + 1])
                            nc.vector.tensor_add(xT[:, m, t0:t0 + tl],
                                                 xT[:, m, t0:t0 + tl], tmp[:, :tl])
                if debug:
                    nc.sync.dma_start(
                        dbg["y_dbg"].rearrange("(k p) t -> p k t", p=128), xT)

        # ============ Phase 3: LN2 + basis1 ============
        bsp = top.enter_context(tc.tile_pool(name="bsp", bufs=1))
        b1t = bsp.tile([128, CT, TOK], F32R)        # T1 = tanh
        b1p2 = bsp.tile([128, CT, TOK], F32R)       # T2
        b1p3 = bsp.tile([128, CT, TOK], F32R)       # T3
        b1planes = (b1t, b1p2, b1p3)
        with ExitStack() as ph:
            ph.enter_context(nc.named_scope("p3_ln2_basis"))
            work = ph.enter_context(tc.tile_pool(name="w3", bufs=1))
            with tc.tile_pool(name="ps_ln2", bufs=1, space="PSUM") as ps_ln2:
                a2, b2b = layernorm_bcast(xT, work, ps_ln2, "ln2")
                for k in range(CT):
                    nsc = work.tile([128, TOK], F32R, tag="nsc", bufs=2, name=f"n{k}")
                    nc.vector.tensor_mul(nsc, xT[:, k, :], a2)
                    nc.vector.tensor_add(nsc, nsc, b2b)
                    nc.vector.tensor_scalar(nsc, nsc, g2_sb[:, k:k + 1],
                                            b2_sb[:, k:k + 1], OP.mult, OP.add)
                    nc.scalar.activation(b1t[:, k, :], nsc, AF.Tanh)
            for k in range(CT):
                t_ = b1t[:, k, :]
                p2 = b1p2[:, k, :]
                p3 = b1p3[:, k, :]
                nc.vector.tensor_mul(p2, t_, t_)
                nc.vector.tensor_scalar(p2, p2, 2.0, -1.0, OP.mult, OP.add)
                nc.vector.tensor_scalar(p3, p2, 2.0, -1.0, OP.mult, OP.add)
                nc.vector.tensor_mul(p3, p3, t_)
            if debug:
                nc.sync.dma_start(
                    dbg["t1_dbg"].rearrange("(k p) t -> p k t", p=128), b1t)
            for m2 in range(CT):    # y += cheby2 d=0 bias, before accumulation
                nc.vector.tensor_scalar_add(xT[:, m2, :], xT[:, m2, :],
                                            b2e_sb[:, m2:m2 + 1])

        # ============ Phase 4: cheby1 -> basis2 -> cheby2 (per hidden group) ============
        with ExitStack() as ph:
            ph.enter_context(nc.named_scope("p4_cheby"))
            gb = ph.enter_context(tc.tile_pool(name="gb", bufs=1))
            tg = gb.tile([128, GT, TOK], F32R)
            p2g = gb.tile([128, GT, TOK], F32R)
            p3g = gb.tile([128, GT, TOK], F32R)
            c1s = ph.enter_context(tc.tile_pool(name="c1s", bufs=2))
            c2s = ph.enter_context(tc.tile_pool(name="c2s", bufs=2))
            ps_c1 = ph.enter_context(tc.tile_pool(name="ps_c1", bufs=3, space="PSUM"))
            ps_c2 = ph.enter_context(tc.tile_pool(name="ps_c2", bufs=3, space="PSUM"))
            for g in range(NG):
                for m in range(GT):
                    cw = c1s.tile([128, 18, 128], F32R, tag="c1w", name=f"c1_{g}_{m}")
                    nc.sync.dma_start(
                        cw, c1e_d[g, m].rearrange("p (k q) -> p k q", q=128))
                    hm = g * GT + m
                    for ch, (t0, tl) in enumerate(TCH):
                        ps = ps_c1.tile([128, TC0], F32, tag="c1",
                                        name=f"pc1_{g}_{m}_{ch}")
                        for k in range(18):
                            d, j = divmod(k, 6)
                            nc.tensor.matmul(ps[:, :tl], cw[:, k, :],
                                             b1planes[d][:, j, t0:t0 + tl],
                                             start=(k == 0), stop=(k == 17))
                        nc.scalar.activation(tg[:, m, t0:t0 + tl], ps[:, :tl], AF.Tanh,
                                             bias=b1e_sb[:, hm:hm + 1])
                if debug:
                    nc.sync.dma_start(
                        dbg["hmid_dbg"].rearrange("(g m p) t -> p g m t",
                                                  p=128, g=NG)[:, g], tg)
                for m in range(GT):
                    t_ = tg[:, m, :]
                    nc.vector.tensor_mul(p2g[:, m, :], t_, t_)
                    nc.vector.tensor_scalar(p2g[:, m, :], p2g[:, m, :], 2.0, -1.0,
                                            OP.mult, OP.add)
                    nc.vector.tensor_scalar(p3g[:, m, :], p2g[:, m, :], 2.0, -1.0,
                                            OP.mult, OP.add)
                    nc.vector.tensor_mul(p3g[:, m, :], p3g[:, m, :], t_)
                planes = (tg, p2g, p3g)
                for m2 in range(CT):
                    cw2 = c2s.tile([128, 18, 128], F32R, tag="c2w", name=f"c2_{g}_{m2}")
                    nc.sync.dma_start(
                        cw2, c2e_d[g, m2].rearrange("p (k q) -> p k q", q=128))
                    for ch, (t0, tl) in enumerate(TCH):
                        ps = ps_c2.tile([128, TC0], F32, tag="c2",
                                        name=f"pc2_{g}_{m2}_{ch}")
                        for k in range(18):
                            d, j = divmod(k, 6)
                            nc.tensor.matmul(ps[:, :tl], cw2[:, k, :],
                                             planes[d][:, j, t0:t0 + tl],
                                             start=(k == 0), stop=(k == 17))
                        nc.vector.tensor_add(xT[:, m2, t0:t0 + tl],
                                             xT[:, m2, t0:t0 + tl], ps[:, :tl])

        nc.sync.dma_start(outT_d.rearrange("(k p) t -> p k t", p=128), xT)

    nc.compile()
    return nc


# ---------------- host-side preprocessing ----------------
def _prep_weights(g1, b1, w_qkv, w_proj, b_proj, g2, b2, c1, c2):
    f32 = np.float32
    wqkv_eff = (g1[:, None] * w_qkv).astype(np.float16)          # [C, 3C] fp16
    bqkv = (b1 @ w_qkv).astype(f32)                              # [3C]
    bqk = np.ascontiguousarray(bqkv[:2 * C].reshape(12, 128).T)  # [128, 12]
    bv = np.ascontiguousarray(bqkv[2 * C:].reshape(1, C))        # [1, C]
    bproj = np.ascontiguousarray(b_proj.reshape(CT, 128).T)      # [128, CT]
    g2p = np.ascontiguousarray(g2.reshape(CT, 128).T)
    b2p = np.ascontiguousarray(b2.reshape(CT, 128).T)
    c1p = np.ascontiguousarray(c1[:, :, 1:4].transpose(2, 0, 1)) # [3, C, HID]
    c1p = c1p.reshape(3, CT, 128, NG, GT, 128)                   # d, j, p, g, m, q
    c1e = np.ascontiguousarray(c1p.transpose(3, 4, 2, 0, 1, 5))  # g, m, p, d, j, q
    c1e = c1e.reshape(NG, GT, 128, 18 * 128)
    b1e = np.ascontiguousarray(c1[:, :, 0].sum(0).reshape(HID // 128, 128).T)
    c2p = np.ascontiguousarray(c2[:, :, 1:4].transpose(2, 0, 1)) # [3, HID, C]
    c2p = c2p.reshape(3, NG, CT, 128, CT, 128)                   # d, g, j, p, m2, q
    c2e = np.ascontiguousarray(c2p.transpose(1, 4, 3, 0, 2, 5))  # g, m2, p, d, j, q
    c2e = c2e.reshape(NG, CT, 128, 18 * 128)
    b2e = np.ascontiguousarray(c2[:, :, 0].sum(0).reshape(CT, 128).T)
    return dict(ones=np.ones((128, 128), f32), wqkv=wqkv_eff, bqk=bqk, bv=bv,
                wproj=np.ascontiguousarray(w_proj), bproj=bproj, g2=g2p, b2=b2p,
                c1e=c1e, b1e=b1e, c2e=c2e, b2e=b2e)


_NC_CACHE = {}


def kernel(x, g1, b1, w_qkv, w_proj, b_proj, g2, b2, c1, c2):
    x = np.asarray(x, np.float32)
    wd = _prep_weights(np.asarray(g1, np.float32), np.asarray(b1, np.float32),
                       np.asarray(w_qkv, np.float32), np.asarray(w_proj, np.float32),
                       np.asarray(b_proj, np.float32), np.asarray(g2, np.float32),
                       np.asarray(b2, np.float32), np.asarray(c1, np.float32),
                       np.asarray(c2, np.float32))
    if "nc" not in _NC_CACHE:
        _NC_CACHE["nc"] = build_nc(debug=False)
    nc = _NC_CACHE["nc"]

    in_maps = []
    for i in range(NCORES):
        xs = x[BL * i:BL * (i + 1)].reshape(TOK, C)
        m = {"xT": np.ascontiguousarray(xs.T)}
        m.update(wd)
        in_maps.append(m)

    from concourse.bass_utils import run_bass_kernel_spmd
    res = run_bass_kernel_spmd(nc, in_maps, core_ids=list(range(NCORES)))
    outs = []
    for i in range(NCORES):
        yT = res.results[i]["outT"]                  # [C, TOK]
        outs.append(yT.T.reshape(BL, N, C))
    return np.concatenate(outs, 0).astype(np.float32)


if __name__ == "__main__":
    build_nc()
    print("built ok")
